# revision 16
# baseline (speedup 1.0000x reference)
"""Batched Kalman filter + RTS smoother on 8 Trainium2 NeuronCores.

Math: P0 is batch-uniform, so the covariance recursion (gains K_t, smoother
gains G_t) is shared across the batch; the smoother covariance recursion does
not affect the returned states. The problem reduces to two linear scans
  forward : sf[t] = sf[t-1]@Mf[t] + u[t]@Wu[t] + y[t]@Wy[t]
  predict : sp[t] = sf[t-1]@F^T + DT*u[t]@Bc^T
  backward: r[t]  = (w[t+1]+r[t+1])@G[t]^T,  w = sf-sp;  ss = sf + r
with shared [16,16] matrices. Time is blocked (k=8) into block-triangular
weights built on the host in float64, so the device runs 16 serial steps per
direction, each one PSUM-accumulated matmul group over a [rows,256] batch
panel, at fp32r full PE rate (moving free size 256).

Data parallel: batch 2048 -> 8 cores x 256. States live transposed [16k, B]
on-chip; host pre-transposes inputs and post-transposes outputs.
"""
import sys

import numpy as np

sys.path.insert(0, "/opt/trn_rl_repo")

DT = 0.01
T, N, M, C = 128, 16, 8, 4
KB = 8            # timesteps per block
NB = T // KB      # 16 blocks
BCORES = 8
BLOC = 2048 // BCORES  # 256 batch per core

TRACE = False          # test.py flips this for profiling
POS = [2, 1, 3, 4, 5, 6, 7, 0]  # pos_of[j]: row-block position of timestep j
LAST_RESULTS = None    # BassKernelResults stash for test.py
MM_DT = "float32r"     # matmul operand dtype


# ---------------------------------------------------------------- host math
def _host_weights(P0_0, A, Bc, H, Q, R):
    f8 = np.float64
    A, Bc, H, Q, R = (x.astype(f8) for x in (A, Bc, H, Q, R))
    I = np.eye(N, dtype=f8)
    F = I + DT * A
    P = P0_0.astype(f8)
    Ks, Pps, Pfs = [], [], []
    for _ in range(T):
        Pp = F @ P @ F.T + Q
        S = H @ Pp @ H.T + R
        K = Pp @ H.T @ np.linalg.inv(S)
        P = Pp - K @ H @ Pp
        Ks.append(K); Pps.append(Pp); Pfs.append(P)
    Gs = [Pfs[t] @ F.T @ np.linalg.inv(Pps[t + 1]) for t in range(T - 1)]

    Mf = np.empty((T, N, N)); Wu = np.empty((T, C, N)); Wy = np.empty((T, M, N))
    for t in range(T):
        J = I - H.T @ Ks[t].T
        Mf[t] = F.T @ J
        Wu[t] = DT * Bc.T @ J
        Wy[t] = Ks[t].T
    Fr = F.T

    def mprod(i, a, b):
        P_ = I.copy()
        for t in range(KB * i + a, KB * i + b + 1):
            P_ = P_ @ Mf[t]
        return P_

    fu = np.zeros((NB, C * KB, N * KB)); fy = np.zeros((NB, M * KB, N * KB))
    fb = np.zeros((NB, N, N * KB))
    pu = np.zeros((NB, C * KB, N * KB)); py = np.zeros((NB, M * KB, N * KB))
    pb = np.zeros((NB, N, N * KB))
    for i in range(NB):
        for j in range(KB):
            cj = POS[j]
            fb[i, :, N * cj:N * (cj + 1)] = mprod(i, 0, j)
            for l in range(j + 1):
                Pl = mprod(i, l + 1, j)
                fu[i, C * l:C * (l + 1), N * cj:N * (cj + 1)] = Wu[KB * i + l] @ Pl
                fy[i, M * l:M * (l + 1), N * cj:N * (cj + 1)] = Wy[KB * i + l] @ Pl
            pb[i, :, N * cj:N * (cj + 1)] = mprod(i, 0, j - 1) @ Fr
            pu[i, C * j:C * (j + 1), N * cj:N * (cj + 1)] += DT * Bc.T
            for l in range(j):
                Pl = mprod(i, l + 1, j - 1)
                pu[i, C * l:C * (l + 1), N * cj:N * (cj + 1)] += Wu[KB * i + l] @ Pl @ Fr
                py[i, M * l:M * (l + 1), N * cj:N * (cj + 1)] = Wy[KB * i + l] @ Pl @ Fr

    Gt = np.concatenate([np.transpose(np.array(Gs), (0, 2, 1)),
                         np.zeros((1, N, N))])  # G[T-1] := 0 handles final block

    def gprod(l, t):
        P_ = I.copy()
        for s in range(l - 1, t - 1, -1):
            P_ = P_ @ Gt[s]
        return P_

    bw = np.zeros((NB, N * KB, N * KB)); bv = np.zeros((NB, N, N * KB))
    for i in range(NB):
        for j in range(KB):
            t = KB * i + j
            cj = POS[j]
            for p in range(j + 1, KB):
                bw[i, N * POS[p]:N * (POS[p] + 1), N * cj:N * (cj + 1)] = gprod(KB * i + p, t)
            bv[i, :, N * cj:N * (cj + 1)] = gprod(KB * (i + 1), t)

    f4 = np.float32
    return {k: np.ascontiguousarray(v, f4) for k, v in
            dict(fu=fu, fy=fy, fb=fb, pu=pu, py=py, pb=pb, bw=bw, bv=bv).items()}


# ---------------------------------------------------------------- device IR
def _build_bass():
    import concourse.bass as bass
    import concourse.mybir as mybir
    import concourse.tile as tile

    fr = getattr(mybir.dt, MM_DT)
    f32 = mybir.dt.float32
    nc = bass.Bass()

    d_ud = nc.dram_tensor("ud", [32, NB * BLOC], fr, kind="ExternalInput")
    d_yd = nc.dram_tensor("yd", [64, NB * BLOC], fr, kind="ExternalInput")
    d_s0 = nc.dram_tensor("s0_t", [N, BLOC], fr, kind="ExternalInput")
    d_w32 = nc.dram_tensor("w32", [32, 2 * NB * 128], fr, kind="ExternalInput")
    d_w64 = nc.dram_tensor("w64", [64, 2 * NB * 128], fr, kind="ExternalInput")
    d_w16 = nc.dram_tensor("w16", [16, 3 * NB * 128], fr, kind="ExternalInput")
    d_w128 = nc.dram_tensor("w128", [128, NB * 128], fr, kind="ExternalInput")
    d_out = nc.dram_tensor("ss_t", [128, NB * BLOC], f32, kind="ExternalOutput")

    with tile.TileContext(nc) as tc:
        with (
            tc.tile_pool(name="persist", bufs=1) as pp,
            tc.tile_pool(name="roll", bufs=4) as roll,
            tc.tile_pool(name="ps_sf", bufs=2, space=bass.MemorySpace.PSUM) as ps_sf,
            tc.tile_pool(name="ps_sp", bufs=2, space=bass.MemorySpace.PSUM) as ps_sp,
            tc.tile_pool(name="ps_r", bufs=2, space=bass.MemorySpace.PSUM) as ps_r,
            tc.tile_pool(name="ps_touch", bufs=1, space=bass.MemorySpace.PSUM) as ps_touch,
        ):
            touch_sc = ps_touch.tile([4, 4], f32, tag="touch", name="touch")

            def load(dram, shape, tag):
                t = pp.tile(list(shape), fr, tag=tag, name=tag)
                nc.gpsimd.dma_start(t[:], dram[:])
                # PE pre-touch: walrus codegen allows only ONE sync wait per
                # instruction; absorb each DMA dependency into a trivial PE
                # matmul so real matmuls never wait on DMA semaphores.
                p = min(shape[0], 32)
                nc.tensor.matmul(touch_sc[:], t[0:p, 0:4], t[0:p, 0:4],
                                 start=True, stop=True, skip_group_check=True)
                return t

            ud = load(d_ud, (32, NB * BLOC), "ud")
            yd = load(d_yd, (64, NB * BLOC), "yd")
            s0_sb = load(d_s0, (N, BLOC), "s0")
            w32 = load(d_w32, (32, 2 * NB * 128), "w32")
            w64 = load(d_w64, (64, 2 * NB * 128), "w64")
            w16 = load(d_w16, (16, 3 * NB * 128), "w16")
            w128 = load(d_w128, (128, NB * 128), "w128")
            SEG = NB * 128

            def seg(t, rows, s, i):
                return t[0:rows, s * SEG + i * 128:s * SEG + (i + 1) * 128]

            sf_sb = [pp.tile([128, BLOC], fr, tag=f"sf{i}", name=f"sf{i}") for i in range(NB)]
            # sp_sb holds the NEGATED prediction so w = sf - sp becomes
            # bw@sf + bw@sp_neg via matmul linearity (no PSUM-reading sub).
            sp_sb = [pp.tile([128, BLOC], fr, tag=f"sp{i}", name=f"sp{i}") for i in range(NB)]
            rr_sb = [pp.tile([128, BLOC], fr, tag=f"rr{i}", name=f"rr{i}") for i in range(NB)]
            ss_sb = pp.tile([128, NB * BLOC], f32, tag="ssm", name="ssm")
            v1_sb = [pp.tile([16, BLOC], fr, tag=f"v1{i}", name=f"v1{i}") for i in range(NB)]

            # --- forward: software-pipelined by one block so bulk matmuls of
            # block i+1 sit in the PE queue while block i waits on its boundary.
            psf, psp, bnds = [None] * NB, [None] * NB, [None] * (NB + 1)
            bnds[0] = s0_sb

            def fwd_bulk(i):
                sf_t = ps_sf.tile([128, BLOC], f32, tag="psf", name="psf")
                sp_t = ps_sp.tile([128, BLOC], f32, tag="psp", name="psp")
                psf[i], psp[i] = sf_t, sp_t
                nc.tensor.matmul(sf_t[:], seg(w32, 32, 0, i), ud[:, i * BLOC:(i + 1) * BLOC], start=True, stop=False)
                nc.tensor.matmul(sf_t[:], seg(w64, 64, 0, i), yd[:, i * BLOC:(i + 1) * BLOC], start=False, stop=False)
                nc.tensor.matmul(sp_t[:], seg(w32, 32, 1, i), ud[:, i * BLOC:(i + 1) * BLOC], start=True, stop=False)
                nc.tensor.matmul(sp_t[:], seg(w64, 64, 1, i), yd[:, i * BLOC:(i + 1) * BLOC], start=False, stop=False)

            def fwd_serial(i):
                bnd = bnds[i][:]
                nc.tensor.matmul(psf[i][:], seg(w16, 16, 0, i), bnd, start=False, stop=True)
                nc.tensor.matmul(psp[i][:], seg(w16, 16, 1, i), bnd, start=False, stop=True)
                nbnd = roll.tile([16, BLOC], fr, tag="bnd", name="bnd")
                nc.vector.tensor_copy(nbnd[:], psf[i][0:16, :])
                bnds[i + 1] = nbnd
                nc.vector.tensor_copy(sf_sb[i][:], psf[i][:])
                nc.vector.tensor_scalar_mul(sp_sb[i][:], psp[i][:], -1.0)

            fwd_bulk(0)
            for i in range(NB):
                if i + 1 < NB:
                    fwd_bulk(i + 1)
                fwd_serial(i)

            # --- backward, same pipelining trick, blocks NB-1 .. 0
            pr = [None] * NB

            def bwd_bulk(i):
                r_t = ps_r.tile([128, BLOC], f32, tag="pr", name="pr")
                pr[i] = r_t
                nc.tensor.matmul(r_t[:], seg(w128, 128, 0, i), sf_sb[i][:], start=True, stop=False)
                nc.tensor.matmul(r_t[:], seg(w128, 128, 0, i), sp_sb[i][:],
                                 start=False, stop=(i == NB - 1))

            def bwd_serial(i):
                if i < NB - 1:
                    nc.tensor.matmul(pr[i][:], seg(w16, 16, 2, i), v1_sb[i + 1][:],
                                     start=False, stop=True)
                nc.vector.tensor_copy(rr_sb[i][:], pr[i][:])
                if i > 0:
                    spv = roll.tile([16, BLOC], fr, tag="spv", name="spv")
                    nc.vector.tensor_scalar_add(spv[:], sp_sb[i][32:48, :], 0.0)
                    nc.vector.tensor_add(v1_sb[i][:], rr_sb[i][32:48, :], sf_sb[i][32:48, :])
                    nc.vector.tensor_add(v1_sb[i][:], v1_sb[i][:], spv[:])
                nc.vector.tensor_add(ss_sb[:, i * BLOC:(i + 1) * BLOC],
                                     rr_sb[i][:], sf_sb[i][:])

            bwd_bulk(NB - 1)
            for i in range(NB - 1, -1, -1):
                if i - 1 >= 0:
                    bwd_bulk(i - 1)
                bwd_serial(i)
            nc.gpsimd.dma_start(d_out[:], ss_sb[:])

    return nc


_NC_CACHE = None


def _split_multiwait_drains(nc):
    """Walrus in this stack accepts only one sync-wait per instruction; the
    Tile tail emits one SP Drain waiting on every active proc. Split it into
    a chain of single-wait Drains (equivalent: empty-pipeline drains)."""
    import json as _json
    raw = nc.to_json_bytes()
    j = _json.loads(raw)
    changed = False
    for f in j["functions"]:
        for bb in f["blocks"]:
            il = bb["instructions"]
            k = 0
            while k < len(il):
                ins = il[k]
                si = ins.get("sync_info") or {}
                waits = si.get("on_wait") or []
                if ins.get("opcode") == "Drain" and len(waits) > 1:
                    pre = []
                    for wi, w in enumerate(waits[:-1]):
                        c = _json.loads(_json.dumps(ins))
                        c["name"] = f"{ins['name']}w{wi}"
                        c["sync_info"] = {"on_wait": [w], "on_update": []}
                        pre.append(c)
                    si["on_wait"] = [waits[-1]]
                    il[k:k] = pre
                    k += len(pre)
                    changed = True
                k += 1
    out = _json.dumps(j).encode()
    return out if changed else raw


def kernel(state0, P0, controls, obs, A, Bc, H, Q, R):
    global _NC_CACHE, LAST_RESULTS
    from concourse.bass_utils import run_bass_kernel_spmd

    state0 = np.asarray(state0, np.float32)
    P0 = np.asarray(P0, np.float32)
    controls = np.asarray(controls, np.float32)
    obs = np.asarray(obs, np.float32)
    assert np.all(P0 == P0[0:1]), "shared-gain path requires batch-uniform P0"
    W = _host_weights(np.asarray(P0[0], np.float64), np.asarray(A), np.asarray(Bc),
                      np.asarray(H), np.asarray(Q), np.asarray(R))

    f4 = np.float32
    wm32 = np.zeros((32, 2 * NB * 128), f4)
    wm64 = np.zeros((64, 2 * NB * 128), f4)
    wm16 = np.zeros((16, 3 * NB * 128), f4)
    wm128 = np.zeros((128, NB * 128), f4)
    SEG = NB * 128
    for i in range(NB):
        wm32[:, i * 128:(i + 1) * 128] = W["fu"][i]
        wm32[:, SEG + i * 128:SEG + (i + 1) * 128] = W["pu"][i]
        wm64[:, i * 128:(i + 1) * 128] = W["fy"][i]
        wm64[:, SEG + i * 128:SEG + (i + 1) * 128] = W["py"][i]
        wm16[:, i * 128:(i + 1) * 128] = W["fb"][i]
        wm16[:, SEG + i * 128:SEG + (i + 1) * 128] = W["pb"][i]
        wm16[:, 2 * SEG + i * 128:2 * SEG + (i + 1) * 128] = W["bv"][i]
        wm128[:, i * 128:(i + 1) * 128] = W["bw"][i]

    in_maps = []
    for r in range(BCORES):
        b0 = r * BLOC
        sl = slice(b0, b0 + BLOC)
        uT = controls[sl].reshape(BLOC, T * C).T.reshape(NB, 32, BLOC)
        yT = obs[sl].reshape(BLOC, T * M).T.reshape(NB, 64, BLOC)
        m = {"ud": np.ascontiguousarray(uT.transpose(1, 0, 2).reshape(32, NB * BLOC), f4),
             "yd": np.ascontiguousarray(yT.transpose(1, 0, 2).reshape(64, NB * BLOC), f4),
             "s0_t": np.ascontiguousarray(state0[sl].T, f4),
             "w32": wm32, "w64": wm64, "w16": wm16, "w128": wm128}
        in_maps.append(m)

    if _NC_CACHE is None:
        _NC_CACHE = _build_bass()
        fixed = _split_multiwait_drains(_NC_CACHE)
        _NC_CACHE.to_json_bytes = lambda: fixed
    res = run_bass_kernel_spmd(_NC_CACHE, in_maps,
                               core_ids=list(range(BCORES)), trace=TRACE)
    LAST_RESULTS = res

    out = np.empty((2048, T, N), np.float32)
    for r in range(BCORES):
        ss = res.results[r]["ss_t"]  # [128, NB*BLOC]: row 16*pos+d, col i*BLOC+b
        sb = ss.reshape(KB, N, NB, BLOC)[POS]  # -> [j, d, i, b]
        out[r * BLOC:(r + 1) * BLOC] = (
            sb.transpose(3, 2, 0, 1).reshape(BLOC, T, N))
    return out
